# revision 9
# baseline (speedup 1.0000x reference)
"""4-level inverse DWT (db4, symmetric-mode coefficient layout) on TRN2.

Contract: kernel(**inputs) takes FULL inputs (B=64, C=16 batch/channel dims),
returns the FULL (64, 16, 16384) float32 reconstruction.

Sharding: B*C = 1024 signals -> 8 cores x 128 SBUF partitions. Each core runs
the whole 4-level synthesis bank on its 128 signals; no communication.

Math (polyphase form of pywt idwt, valid for this geometry -- no boundary
handling needed): with filter h (rec_lo for the approx branch, rec_hi for the
detail branch), each level computes
    y[2i+p] = sum_{j=0..3} h[(6+p) - 2j] * x[i+j],  i in [0, n-4]
summed over both branches, where n is the (trimmed) coefficient length.
Output length 2n-6; level sizes 1030->2054->4102(->4101)->8196(->8195)->16384.

Engine split (per parity, 8 taps): the first _PE_TAPS taps run as scaled-
diagonal fp16 matmuls accumulating into a PSUM bank (TensorE does the shifted
multiply-accumulate along the free dim, one 512-col chunk at a time). ScalarE
evacuates PSUM to an fp16 accumulator; VectorE applies the remaining taps as
in-place fp16 scalar_tensor_tensor ops (2x packed mode). The parity interleave
into the level output is written by ScalarE (levels 1-3, fp16 output) or fused
into VectorE's last tap as a strided fp32 write (final level). All inputs are
loaded as fp16 via casting DMAs on the gpsimd SWDGE queues.
"""

import numpy as np

_P = 128
_N_CORES = 8
_IN_LENS = {"approx": 1030, "d0": 1030, "d1": 2054, "d2": 4101, "d3": 8195}
_OUT_LEN = 16384
_CHUNK = 512  # PSUM bank = 512 fp32

_PE_TAPS = 5  # taps per parity on TensorE (0 => pure-DVE fp32 kernel)

# Set by a driving harness (test.py) to collect profile info; harmless default.
_TRACE = False
_LAST_RESULTS = None

_CACHE = {}


def _ensure_paths():
    import sys

    for p in ("/opt/trn_rl_repo", "/root/.axon_site"):
        if p not in sys.path:
            sys.path.insert(0, p)


def _tap_table(lo, hi, p):
    """8 (branch, offset, coef) taps for output parity p, PE-friendly order:
    a-branch taps first, then d-branch."""
    taps = [("a", j, lo[6 + p - 2 * j]) for j in range(4)]
    taps += [("d", j, hi[6 + p - 2 * j]) for j in range(4)]
    return taps


# level -> (d name, common length n after any trim of the approx input)
_LEVELS = [("d0", 1030), ("d1", 2054), ("d2", 4101), ("d3", 8195)]


def _build_hybrid(lo, hi, pe_taps):
    """TensorE/VectorE/ScalarE hybrid kernel (fp16 data path)."""
    import concourse.tile as tile
    from concourse import bacc, mybir

    f32 = mybir.dt.float32
    f16 = mybir.dt.float16
    mult = mybir.AluOpType.mult
    add = mybir.AluOpType.add

    nc = bacc.Bacc("TRN2", target_bir_lowering=False, debug=False)

    ins = {
        name: nc.dram_tensor(name, [_P, L], f32, kind="ExternalInput").ap()
        for name, L in _IN_LENS.items()
    }
    n_diags = 2 * pe_taps
    diag_ap = nc.dram_tensor("diag", [_P, n_diags * _P], f16, kind="ExternalInput").ap()
    out_ap = nc.dram_tensor("out", [_P, _OUT_LEN], f32, kind="ExternalOutput").ap()

    with tile.TileContext(nc) as tc:
        with (
            tc.tile_pool(name="bufs", bufs=1) as pool,
            tc.tile_pool(name="tmps", bufs=8) as tmp_pool,
            tc.tile_pool(name="psum", bufs=8, space="PSUM") as ps_pool,
        ):
            # All compute inputs are fp16; load via casting DMAs (gpsimd
            # SWDGE), level-1 inputs first so compute starts early. The
            # (d0,d2) and (d1,d3) pairs share SBUF slots (disjoint lifetimes).
            a_h = pool.tile([_P, 1030], f16, tag="a4h")
            nc.gpsimd.dma_start(a_h[:], ins["approx"])
            dtag = {"d0": "dA", "d1": "dB", "d2": "dA", "d3": "dB"}
            d16 = {}
            for name in ("d0", "d1", "d2", "d3"):
                t16 = pool.tile([_P, _IN_LENS[name]], f16, tag=dtag[name])
                nc.gpsimd.dma_start(t16[:], ins[name])
                d16[name] = t16
            diag = pool.tile([_P, n_diags * _P], f16, tag="diag")
            nc.sync.dma_start(diag[:], diag_ap)

            for lvl, (dname, n) in enumerate(_LEVELS):
                m = n - 3
                last = lvl == len(_LEVELS) - 1
                ot = pool.tile([_P, 2 * m], f32 if last else f16, tag=f"lv{lvl}")
                dh = d16[dname]

                chunks = [(c0, min(_CHUNK, m - c0)) for c0 in range(0, m, _CHUNK)]
                # Tap-outer groups: each diag weight serves G matmuls in a row.
                G = 3
                for g0 in range(0, len(chunks), G):
                    grp = chunks[g0 : g0 + G]
                    pss = {}
                    for ci in range(len(grp)):
                        for p in (0, 1):
                            t_ps = ps_pool.tile([_P, _CHUNK], f32, tag="ps")
                            pss[(ci, p)] = t_ps
                    for k in range(pe_taps):
                        for p in (0, 1):
                            br, j, _c = _tap_table(lo, hi, p)[k]
                            src = a_h if br == "a" else dh
                            di = p * pe_taps + k
                            w = diag[:, di * _P : (di + 1) * _P]
                            for ci, (c0, N) in enumerate(grp):
                                nc.tensor.matmul(
                                    pss[(ci, p)][:, :N],
                                    w,
                                    src[:, c0 + j : c0 + j + N],
                                    start=(k == 0),
                                    stop=(k == pe_taps - 1),
                                )
                    for ci, (c0, N) in enumerate(grp):
                        for p in (0, 1):
                            dve_taps = _tap_table(lo, hi, p)[pe_taps:]
                            # ScalarE: PSUM -> fp16 accumulator
                            acc_t = tmp_pool.tile([_P, _CHUNK], f16, tag="acc")
                            acc = acc_t[:, :N]
                            nc.scalar.copy(acc, pss[(ci, p)][:, :N])
                            out_slice = ot[:, 2 * c0 + p : 2 * (c0 + N) + p - 1 : 2]
                            for k, (_br, j, c) in enumerate(dve_taps):
                                in0 = dh[:, c0 + j : c0 + j + N]
                                final = k == len(dve_taps) - 1
                                if final and last:
                                    # fp32 strided write straight from the
                                    # fp32-internal DVE pipe
                                    nc.vector.scalar_tensor_tensor(
                                        out_slice, in0, c, acc, mult, add
                                    )
                                else:
                                    nc.vector.scalar_tensor_tensor(
                                        acc, in0, c, acc, mult, add
                                    )
                            if not last:
                                # ScalarE writes the parity interleave
                                nc.scalar.copy(out_slice, acc)

                        if last:
                            nc.sync.dma_start(
                                out_ap[:, 2 * c0 : 2 * (c0 + N)],
                                ot[:, 2 * c0 : 2 * (c0 + N)],
                            )

                if not last:
                    a_h = ot

    nc.compile()
    return nc


def _build_dve(lo, hi):
    """Pure-DVE fp32 reference kernel (slow but exact); kept as fallback."""
    import concourse.tile as tile
    from concourse import bacc, mybir

    f32 = mybir.dt.float32
    mult = mybir.AluOpType.mult
    add = mybir.AluOpType.add

    nc = bacc.Bacc("TRN2", target_bir_lowering=False, debug=False)
    ins = {
        name: nc.dram_tensor(name, [_P, L], f32, kind="ExternalInput").ap()
        for name, L in _IN_LENS.items()
    }
    out_ap = nc.dram_tensor("out", [_P, _OUT_LEN], f32, kind="ExternalOutput").ap()

    with tile.TileContext(nc) as tc:
        with tc.tile_pool(name="bufs", bufs=1) as pool:
            tiles = {}
            for name, L in _IN_LENS.items():
                t = pool.tile([_P, L], f32, tag=name)
                nc.sync.dma_start(t[:], ins[name])
                tiles[name] = t
            a = tiles["approx"]
            for lvl, (dname, n) in enumerate(_LEVELS):
                d = tiles[dname]
                m = n - 3
                ot = pool.tile([_P, 2 * m], f32, tag=f"lv{lvl}")
                for p in (0, 1):
                    acc = ot[:, p::2]
                    taps = _tap_table(lo, hi, p)
                    nc.vector.tensor_scalar_mul(acc, a[:, 0:m], taps[0][2])
                    for br, j, c in taps[1:]:
                        src = a if br == "a" else d
                        nc.vector.scalar_tensor_tensor(
                            acc, src[:, j : j + m], c, acc, mult, add
                        )
                a = ot
            nc.sync.dma_start(out_ap, a[:])
    nc.compile()
    return nc


def kernel(approx, d0, d1, d2, d3, rec_lo, rec_hi):
    _ensure_paths()
    global _LAST_RESULTS
    from concourse.bass_utils import run_bass_kernel_spmd

    lo = [float(v) for v in np.asarray(rec_lo, np.float32)]
    hi = [float(v) for v in np.asarray(rec_hi, np.float32)]
    key = (tuple(lo), tuple(hi), _PE_TAPS)
    if key not in _CACHE:
        if _PE_TAPS:
            _CACHE[key] = _build_hybrid(lo, hi, _PE_TAPS)
        else:
            _CACHE[key] = _build_dve(lo, hi)
    nc = _CACHE[key]

    arrs = {"approx": approx, "d0": d0, "d1": d1, "d2": d2, "d3": d3}
    flat = {}
    B, C = None, None
    for name, x in arrs.items():
        x = np.asarray(x, np.float32)
        B, C = x.shape[0], x.shape[1]
        flat[name] = np.ascontiguousarray(x.reshape(B * C, x.shape[-1]))

    in_maps = [
        {name: v[i * _P : (i + 1) * _P] for name, v in flat.items()}
        for i in range(_N_CORES)
    ]
    if _PE_TAPS:
        dg = np.zeros((_P, 2 * _PE_TAPS * _P), np.float16)
        eye = np.eye(_P, dtype=np.float64)
        for p in (0, 1):
            for k, (_br, _j, c) in enumerate(_tap_table(lo, hi, p)[:_PE_TAPS]):
                di = p * _PE_TAPS + k
                dg[:, di * _P : (di + 1) * _P] = (eye * c).astype(np.float16)
        for im in in_maps:
            im["diag"] = dg

    res = run_bass_kernel_spmd(nc, in_maps, list(range(_N_CORES)), trace=_TRACE)
    _LAST_RESULTS = res
    out = np.concatenate([res.results[i]["out"] for i in range(_N_CORES)], axis=0)
    return np.ascontiguousarray(out.reshape(B, C, _OUT_LEN).astype(np.float32))


# revision 12
# speedup vs baseline: 1.3216x; 1.3216x over previous
"""4-level inverse DWT (db4, symmetric-mode coefficient layout) on TRN2.

Contract: kernel(**inputs) takes FULL inputs (B=64, C=16 batch/channel dims),
returns the FULL (64, 16, 16384) float32 reconstruction.

Sharding: B*C = 1024 signals -> 8 cores x 128 SBUF partitions. Each core runs
the whole 4-level synthesis bank on its 128 signals; no communication.

Math (polyphase form of pywt idwt, valid for this geometry -- no boundary
handling needed): with filter h (rec_lo for the approx branch, rec_hi for the
detail branch), each level computes
    y[2i+p] = sum_{j=0..3} h[(6+p) - 2j] * x[i+j],  i in [0, n-4]
summed over both branches, where n is the (trimmed) coefficient length.
Output length 2n-6; level sizes 1030->2054->4102(->4101)->8196(->8195)->16384.

Engine split (per parity, 8 taps): the first _PE_TAPS taps run as scaled-
diagonal fp16 matmuls accumulating into a PSUM bank (TensorE does the shifted
multiply-accumulate along the free dim, one 512-col chunk at a time). ScalarE
evacuates PSUM to an fp16 accumulator; VectorE applies the remaining taps as
in-place fp16 scalar_tensor_tensor ops (2x packed mode). The parity interleave
into the level output is written by ScalarE (levels 1-3, fp16 output) or fused
into VectorE's last tap as a strided fp32 write (final level). All inputs are
loaded as fp16 via casting DMAs on the gpsimd SWDGE queues.
"""

import numpy as np

_P = 128
_N_CORES = 8
_IN_LENS = {"approx": 1030, "d0": 1030, "d1": 2054, "d2": 4101, "d3": 8195}
_OUT_LEN = 16384
_CHUNK = 512  # PSUM bank = 512 fp32

_PE_TAPS = 6  # taps per parity on TensorE (0 => pure-DVE fp32 kernel)

# Set by a driving harness (test.py) to collect profile info; harmless default.
_TRACE = False
_LAST_RESULTS = None

_CACHE = {}


def _ensure_paths():
    import sys

    for p in ("/opt/trn_rl_repo", "/root/.axon_site"):
        if p not in sys.path:
            sys.path.insert(0, p)


def _tap_table(lo, hi, p):
    """8 (branch, offset, coef) taps for output parity p, PE-friendly order:
    a-branch taps first, then d-branch."""
    taps = [("a", j, lo[6 + p - 2 * j]) for j in range(4)]
    taps += [("d", j, hi[6 + p - 2 * j]) for j in range(4)]
    return taps


# level -> (d name, common length n after any trim of the approx input)
_LEVELS = [("d0", 1030), ("d1", 2054), ("d2", 4101), ("d3", 8195)]


def _build_hybrid(lo, hi, pe_taps):
    """TensorE (fp16 diag matmuls) + VectorE (fp32 STT endpoint taps)."""
    import concourse.tile as tile
    from concourse import bacc, mybir

    f32 = mybir.dt.float32
    f16 = mybir.dt.float16
    mult = mybir.AluOpType.mult
    add = mybir.AluOpType.add

    nc = bacc.Bacc("TRN2", target_bir_lowering=False, debug=False)

    # fp16 copies of every input are prepared host-side (half the load
    # bytes, no on-chip casts); fp32 details stay for the DVE taps.
    ins32 = {
        name: nc.dram_tensor(name, [_P, L], f32, kind="ExternalInput").ap()
        for name, L in _IN_LENS.items()
        if name != "approx"
    }
    ins16 = {
        name: nc.dram_tensor(f"{name}16", [_P, L], f16, kind="ExternalInput").ap()
        for name, L in _IN_LENS.items()
    }
    n_diags = 2 * pe_taps
    diag_ap = nc.dram_tensor("diag", [_P, n_diags * _P], f16, kind="ExternalInput").ap()
    out_ap = nc.dram_tensor("out", [_P, _OUT_LEN], f32, kind="ExternalOutput").ap()

    with tile.TileContext(nc) as tc:
        with (
            tc.tile_pool(name="bufs", bufs=1) as pool,
            tc.tile_pool(name="tmps", bufs=8) as tmp_pool,
            tc.tile_pool(name="psum", bufs=8, space="PSUM") as ps_pool,
        ):
            # Level-1 inputs first so compute starts early. (d0,d2) and
            # (d1,d3) share SBUF slots (disjoint lifetimes) in both dtypes.
            dtag = {"d0": "dA", "d1": "dB", "d2": "dA", "d3": "dB"}
            a_h = pool.tile([_P, 1030], f16, tag="a4h")
            nc.sync.dma_start(a_h[:], ins16["approx"])
            d16, d32 = {}, {}

            def load_d(name):
                t16 = pool.tile([_P, _IN_LENS[name]], f16, tag=dtag[name] + "h")
                nc.sync.dma_start(t16[:], ins16[name])
                d16[name] = t16
                t32 = pool.tile([_P, _IN_LENS[name]], f32, tag=dtag[name])
                nc.sync.dma_start(t32[:], ins32[name])
                d32[name] = t32

            load_d("d0")
            diag = pool.tile([_P, n_diags * _P], f16, tag="diag")
            nc.sync.dma_start(diag[:], diag_ap)
            for name in ("d1", "d2", "d3"):
                load_d(name)

            for lvl, (dname, n) in enumerate(_LEVELS):
                m = n - 3
                last = lvl == len(_LEVELS) - 1
                ot = pool.tile([_P, 2 * m], f32 if last else f16, tag=f"lv{lvl}")
                dh = d16[dname]
                df = d32[dname]

                chunks = [(c0, min(_CHUNK, m - c0)) for c0 in range(0, m, _CHUNK)]
                # Emit in groups; smaller trailing groups on the last level
                # shorten the DVE tail after the final matmul.
                G = 2
                starts = list(range(0, len(chunks), G))
                groups = [chunks[g0 : g0 + G] for g0 in starts]
                if last and len(groups) >= 2 and len(groups[-1]) == G:
                    tail = groups.pop()
                    groups += [[c] for c in tail]
                for grp in groups:
                    pss = {}
                    for ci in range(len(grp)):
                        for p in (0, 1):
                            t_ps = ps_pool.tile([_P, _CHUNK], f32, tag="ps")
                            pss[(ci, p)] = t_ps
                    for k in range(pe_taps):
                        for p in (0, 1):
                            br, j, _c = _tap_table(lo, hi, p)[k]
                            src = a_h if br == "a" else dh
                            di = p * pe_taps + k
                            w = diag[:, di * _P : (di + 1) * _P]
                            for ci, (c0, N) in enumerate(grp):
                                nc.tensor.matmul(
                                    pss[(ci, p)][:, :N],
                                    w,
                                    src[:, c0 + j : c0 + j + N],
                                    start=(k == 0),
                                    stop=(k == pe_taps - 1),
                                )
                    for ci, (c0, N) in enumerate(grp):
                        for p in (0, 1):
                            dve_taps = _tap_table(lo, hi, p)[pe_taps:]
                            acc = pss[(ci, p)][:, :N]
                            out_slice = ot[:, 2 * c0 + p : 2 * (c0 + N) + p - 1 : 2]
                            for k, (_br, j, c) in enumerate(dve_taps):
                                in0 = df[:, c0 + j : c0 + j + N]
                                final = k == len(dve_taps) - 1
                                if final:
                                    dst = out_slice
                                else:
                                    tmp_t = tmp_pool.tile([_P, _CHUNK], f32, tag="t")
                                    dst = tmp_t[:, :N]
                                nc.vector.scalar_tensor_tensor(
                                    dst, in0, c, acc, mult, add
                                )
                                acc = dst

                        if last:
                            nc.sync.dma_start(
                                out_ap[:, 2 * c0 : 2 * (c0 + N)],
                                ot[:, 2 * c0 : 2 * (c0 + N)],
                            )

                if not last:
                    a_h = ot

    nc.compile()
    return nc


def _build_dve(lo, hi):
    """Pure-DVE fp32 reference kernel (slow but exact); kept as fallback."""
    import concourse.tile as tile
    from concourse import bacc, mybir

    f32 = mybir.dt.float32
    mult = mybir.AluOpType.mult
    add = mybir.AluOpType.add

    nc = bacc.Bacc("TRN2", target_bir_lowering=False, debug=False)
    ins = {
        name: nc.dram_tensor(name, [_P, L], f32, kind="ExternalInput").ap()
        for name, L in _IN_LENS.items()
    }
    out_ap = nc.dram_tensor("out", [_P, _OUT_LEN], f32, kind="ExternalOutput").ap()

    with tile.TileContext(nc) as tc:
        with tc.tile_pool(name="bufs", bufs=1) as pool:
            tiles = {}
            for name, L in _IN_LENS.items():
                t = pool.tile([_P, L], f32, tag=name)
                nc.sync.dma_start(t[:], ins[name])
                tiles[name] = t
            a = tiles["approx"]
            for lvl, (dname, n) in enumerate(_LEVELS):
                d = tiles[dname]
                m = n - 3
                ot = pool.tile([_P, 2 * m], f32, tag=f"lv{lvl}")
                for p in (0, 1):
                    acc = ot[:, p::2]
                    taps = _tap_table(lo, hi, p)
                    nc.vector.tensor_scalar_mul(acc, a[:, 0:m], taps[0][2])
                    for br, j, c in taps[1:]:
                        src = a if br == "a" else d
                        nc.vector.scalar_tensor_tensor(
                            acc, src[:, j : j + m], c, acc, mult, add
                        )
                a = ot
            nc.sync.dma_start(out_ap, a[:])
    nc.compile()
    return nc


def kernel(approx, d0, d1, d2, d3, rec_lo, rec_hi):
    _ensure_paths()
    global _LAST_RESULTS
    from concourse.bass_utils import run_bass_kernel_spmd

    lo = [float(v) for v in np.asarray(rec_lo, np.float32)]
    hi = [float(v) for v in np.asarray(rec_hi, np.float32)]
    key = (tuple(lo), tuple(hi), _PE_TAPS)
    if key not in _CACHE:
        if _PE_TAPS:
            _CACHE[key] = _build_hybrid(lo, hi, _PE_TAPS)
        else:
            _CACHE[key] = _build_dve(lo, hi)
    nc = _CACHE[key]

    arrs = {"approx": approx, "d0": d0, "d1": d1, "d2": d2, "d3": d3}
    flat = {}
    B, C = None, None
    for name, x in arrs.items():
        x = np.asarray(x, np.float32)
        B, C = x.shape[0], x.shape[1]
        flat[name] = np.ascontiguousarray(x.reshape(B * C, x.shape[-1]))

    flat16 = {f"{name}16": v.astype(np.float16) for name, v in flat.items()}
    if _PE_TAPS:
        del flat["approx"]  # fp32 approx is unused by the hybrid kernel
        flat.update(flat16)
    in_maps = [
        {name: v[i * _P : (i + 1) * _P] for name, v in flat.items()}
        for i in range(_N_CORES)
    ]
    if _PE_TAPS:
        dg = np.zeros((_P, 2 * _PE_TAPS * _P), np.float16)
        eye = np.eye(_P, dtype=np.float64)
        for p in (0, 1):
            for k, (_br, _j, c) in enumerate(_tap_table(lo, hi, p)[:_PE_TAPS]):
                di = p * _PE_TAPS + k
                dg[:, di * _P : (di + 1) * _P] = (eye * c).astype(np.float16)
        for im in in_maps:
            im["diag"] = dg

    res = run_bass_kernel_spmd(nc, in_maps, list(range(_N_CORES)), trace=_TRACE)
    _LAST_RESULTS = res
    out = np.concatenate([res.results[i]["out"] for i in range(_N_CORES)], axis=0)
    return np.ascontiguousarray(out.reshape(B, C, _OUT_LEN).astype(np.float32))
